# revision 31
# baseline (speedup 1.0000x reference)
"""Trainium2 Bass kernel for nn_ActorCritic_38886633898257.

Computes, for each batch row b of x (B, S, E):
  pairs[t]  = concat(x[b, t], x[b, t+1])            t in [0, S-2]
  h         = relu(pairs @ W1 + b1)
  scores[t] = h @ W2[:, 0]                          (+ b2, shift-invariant)
  logits    = scores masked to t < len_b - 1
  logp      = log_softmax(logits)
  out[b]    = (logp[action_b], entropy(logits))

Strategy: pure data parallel over 8 NeuronCores (32 rows each), rows
length-sorted and dealt round-robin so all cores share one compile-time
padded length profile (~3% padded work).

v2 dataflow (vs the per-slot baseline): all 32 slots' positions are packed
into ONE long column axis (P = sum slot_len columns, exact lengths, no
per-slot rounding). x is stored as two 128-feature planes, host-transposed
to (128, 2, cols) fp8 e4m3 so DMA loads need no xbar transpose and the
"x[t+1]" operand is a free +1 column shift; slot-boundary columns compute
garbage scores that are simply never scattered out.

mm1 runs in fp8 DoubleRow mode: stationary = W1 chunk pairs
[128, 2, 128] (x64 scale to dodge e4m3 denormals, rescaled via w2/64),
moving = plane-pair AP [128, 2, N] of the packed x, K=256 per pass ->
2 matmuls per (512-col tile, 128-out block) instead of 4 bf16 ones.
Loop order is weight-major over 3-tile sweeps so each stationary operand
is reused across 3 consecutive N=512 matmuls (amortizing LDWEIGHTS, which
hardware does not hide for full-array matmuls).

relu evacuates psum pair-banks (2 banks per ACT/DVE op) into a packed
bf16 h (128, 4, cols-per-sweep). mm2 = per-slot column slices of h,
4 accumulated [128,1]-stationary matmuls in one of 4 PE column groups
(4 slots run concurrently), then staged + scattered into a (32, 512)
scores matrix. One batched masked-softmax block (fp32) computes
logp-at-action and entropy for all 32 rows and DMAs out (32, 2).
"""

import numpy as np
import ml_dtypes
from contextlib import ExitStack

import concourse.bass as bass
import concourse.tile as tile
from concourse import mybir
from concourse.bass_utils import run_bass_kernel_spmd
import bass_rust

F32 = mybir.dt.float32
BF16 = mybir.dt.bfloat16
FP8 = mybir.dt.float8e4
NP_BF16 = ml_dtypes.bfloat16
NP_FP8 = ml_dtypes.float8_e4m3  # TRN e4m3 variant (max +-240)
N_CORES = 8
B, S, E = 256, 512, 256
BC = B // N_CORES  # rows per core
NEG = -1e9
W1SCALE = 64.0     # fp8 weight pre-scale (undone via w2/64)

TILE_W = 512       # mm1 column tile (one psum bank)
SWEEP_T = 6        # tiles per sweep (weight-reuse run; psum: 6+1+1=8 banks)
SW_W = TILE_W * SWEEP_T
CW = SW_W + 16     # allocated sweep width (+1 shift col, %16 for DR strides)

KNOBS = {
    "mode": "dr",        # 'dr' (fp8 DoubleRow) | 'bf16'
    "mm2_group": 4,      # slots per col-tiled mm2 batch
    "hps_bufs": 6,
    "scps_bufs": 1,
    "h_bufs": 3,
    "xt_bufs": 4,
    "no_mm2": False,     # timing probe: skip mm2/stage/scatter
    "no_stage": False,   # timing probe: mm2 matmuls but no stage/scatter
    "junk_n": 64,        # junk matmul width (barrier only; see below)
    "junk": True,        # sacrificial DR matmul per weight switch: absorbs
                         # the pulled-ahead next LDWEIGHTS, which otherwise
                         # clobbers the in-flight DR matmul's weight state
}

AF = mybir.ActivationFunctionType
ALU = mybir.AluOpType
AX = mybir.AxisListType
DR = mybir.MatmulPerfMode.DoubleRow


def _geom(slot_len):
    """Packed-column geometry: slot bases, total cols, tiles, sweeps."""
    bases = [0]
    for L in slot_len:
        bases.append(bases[-1] + int(L))
    P = bases[-1]
    ntile = -(-P // TILE_W)
    nsweep = -(-ntile // SWEEP_T)
    return bases[:-1], P, ntile, nsweep


# --------------------------------------------------------------------------
# walrus in this toolchain rejects instructions with more than one sync wait
# ("Too many sync wait commands"); split extras onto preceding same-engine
# NOP carriers.
_MAXW = 1


def _split_sync_waits(nc):
    for bb in nc.main_func.blocks:
        il = bb.instructions
        i = 0
        while i < len(il):
            ins = il[i]
            si = ins.sync_info
            if si is not None and len(si.on_wait) > _MAXW:
                waits = list(si.on_wait)
                keep, rest = waits[-_MAXW:], waits[:-_MAXW]
                ins.sync_info = bass_rust.SyncInfo(
                    on_wait=keep, on_update=list(si.on_update))
                carriers = []
                for k in range(0, len(rest), _MAXW):
                    nop = mybir.InstNoOp(
                        name=f"waitsplit-{nc.next_id()}", ins=[], outs=[])
                    nop.engine = ins.engine
                    nop.sync_info = bass_rust.SyncInfo(
                        on_wait=rest[k:k + _MAXW], on_update=[])
                    carriers.append(nop)
                for j, nop in enumerate(carriers):
                    il.insert(i + j, nop)
                i += len(carriers)
            i += 1


# --------------------------------------------------------------------------
def _build_program(slot_len, zero_b1=True, repeat=1, debug_scores=False):
    """Emit the SPMD program. slot_len: 32 compile-time padded lengths."""
    nc = bass.Bass()
    bases, P, ntile, nsweep = _geom(slot_len)
    mode = KNOBS["mode"]
    sc_d = None
    if debug_scores:
        sc_d = nc.declare_dram_parameter("scdbg", [BC, S], F32, isOutput=True)

    x_d = nc.declare_dram_parameter("x", [nsweep, 128, 2, CW],
                                    FP8 if mode == "dr" else BF16,
                                    isOutput=False)
    w1_d = nc.declare_dram_parameter("w1", [128, 16, 128],
                                     FP8 if mode == "dr" else BF16,
                                     isOutput=False)
    w2_d = nc.declare_dram_parameter("w2", [128, 4], BF16, isOutput=False)
    b1_d = nc.declare_dram_parameter("b1", [128, 4], F32, isOutput=False)
    mb_d = nc.declare_dram_parameter("maskbias", [BC, S], F32, isOutput=False)
    oh_d = nc.declare_dram_parameter("onehot", [BC, S], F32, isOutput=False)
    out_d = nc.declare_dram_parameter("out", [BC, 2], F32, isOutput=True)

    with ExitStack() as ctx:
        tc = ctx.enter_context(tile.TileContext(nc))
        singles = ctx.enter_context(tc.tile_pool(name="singles", bufs=1))
        xt_p = ctx.enter_context(tc.tile_pool(name="xt", bufs=KNOBS["xt_bufs"]))
        hps_p = ctx.enter_context(
            tc.tile_pool(name="hps", bufs=KNOBS["hps_bufs"], space="PSUM"))
        h_p = ctx.enter_context(tc.tile_pool(name="h", bufs=KNOBS["h_bufs"]))
        scps_p = ctx.enter_context(
            tc.tile_pool(name="scps", bufs=KNOBS["scps_bufs"], space="PSUM"))
        junk_p = ctx.enter_context(
            tc.tile_pool(name="junk", bufs=1, space="PSUM"))
        stage_p = ctx.enter_context(tc.tile_pool(name="stage", bufs=10))
        sm_p = ctx.enter_context(tc.tile_pool(name="sm", bufs=1))

        # --- one-time loads -----------------------------------------------
        w1_sb = singles.tile([128, 16, 128], FP8 if mode == "dr" else BF16)
        nc.sync.dma_start(out=w1_sb, in_=w1_d[:, :, :])
        w2_sb = singles.tile([128, 4], BF16)
        nc.sync.dma_start(out=w2_sb, in_=w2_d[:, :])
        b1_sb = singles.tile([128, 4], F32)
        nc.sync.dma_start(out=b1_sb, in_=b1_d[:, :])
        mb_sb = singles.tile([BC, S], F32)
        nc.sync.dma_start(out=mb_sb, in_=mb_d[:, :])
        oh_sb = singles.tile([BC, S], F32)
        nc.sync.dma_start(out=oh_sb, in_=oh_d[:, :])

        # Pull the exp/ln activation tables in early so the ~2.7us table DMA
        # overlaps the main pipeline instead of landing in the tail.
        warm = singles.tile([1, 2], F32)
        nc.vector.memset(warm, 1.0)
        nc.scalar.activation(warm[:, 0:1], warm[:, 0:1], AF.Exp)
        nc.scalar.activation(warm[:, 1:2], warm[:, 1:2], AF.Ln)

        scores_all = singles.tile([BC, S], F32, tag="sa")
        # one-time zero: scatters fully overwrite [0:TL_j] every iteration
        # and only the tail columns need the zeros
        nc.vector.memset(scores_all, 0.0)

        args = (slot_len, bases, P, ntile, nsweep, zero_b1,
                x_d, w1_sb, w2_sb, b1_sb,
                xt_p, hps_p, h_p, scps_p, junk_p, stage_p, scores_all)
        if repeat > 1:
            with tc.For_i(0, repeat, 1, hint_engines=(mybir.EngineType.PE,)):
                _emit_rep(nc, *args)
        else:
            _emit_rep(nc, *args)
        if debug_scores:
            nc.sync.dma_start(out=sc_d[:, :], in_=scores_all)
        _emit_softmax(nc, sm_p, scores_all, mb_sb, oh_sb, out_d)

    _split_sync_waits(nc)
    return nc


def _emit_mm2_batch(nc, batch, scps_p, stage_p, w2_sb, scores_all, par):
    """mm2 for up to 4 slots: one PSUM bank, each slot in its own
    32-partition column group -> the matmuls run concurrently on PE.
    batch: list of (j, TL, pieces); pieces = (h_tile, loc, n, out_off).
    A sweep-crossing slot's pieces go on SEPARATE psum partition rows
    (32s + piece index): a start=True matmul clears has_written for the
    whole partition row, so co-resident pieces would wreck each other's
    accumulation."""
    sc_ps = scps_p.tile([128, 512], F32, tag="scps")
    flat = []  # (col_group, j, TL, ht, loc, n, oo)
    s = 0
    for (j, TL, pieces) in batch:
        for (ht, loc, n, oo) in pieces:
            flat.append((s, j, TL, ht, loc, n, oo))
            s += 1
    assert s <= 4
    for g in range(4):
        for (s, j, TL, ht, loc, n, oo) in flat:
            nc.tensor.matmul(sc_ps[32 * s:32 * s + 1, 0:n],
                             w2_sb[:, g:g + 1], ht[:, g, loc:loc + n],
                             start=(g == 0), stop=(g == 3),
                             tile_position=(0, 32 * s),
                             skip_group_check=True)
    if KNOBS["no_stage"]:
        return
    si = 0
    for bi, (j, TL, pieces) in enumerate(batch):
        stg = stage_p.tile([1, 512], F32, tag="stage")
        for (ht, loc, n, oo) in pieces:
            if (par + bi) % 2 == 0:
                nc.scalar.copy(out=stg[0:1, oo:oo + n],
                               in_=sc_ps[32 * si:32 * si + 1, 0:n])
            else:
                nc.vector.tensor_copy(out=stg[0:1, oo:oo + n],
                                      in_=sc_ps[32 * si:32 * si + 1, 0:n])
            si += 1
        nc.sync.dma_start(out=scores_all[j:j + 1, 0:TL],
                          in_=stg[0:1, 0:TL])


def _emit_rep(nc, slot_len, bases, P, ntile, nsweep, zero_b1,
              x_d, w1_sb, w2_sb, b1_sb,
              xt_p, hps_p, h_p, scps_p, junk_p, stage_p, scores_all):
    mode = KNOBS["mode"]
    GRP = KNOBS["mm2_group"]

    # chunk DMAs: one per sweep, host-pretransposed planes, no xbar
    xts = []
    for sw in range(nsweep):
        xt = xt_p.tile([128, 2, CW], FP8 if mode == "dr" else BF16,
                       tag=f"xt{sw}")
        nc.sync.dma_start(out=xt, in_=x_d[sw])
        xts.append(xt)

    # per-slot mm2 pieces, split at sweep boundaries
    slot_pieces = []   # j -> list of (sweep, loc, n, out_off)
    for j in range(BC):
        TL = int(slot_len[j]) - 1
        b0 = bases[j]
        pieces = []
        c = b0
        while c < b0 + TL:
            sw = c // SW_W
            hi = min(b0 + TL, (sw + 1) * SW_W)
            pieces.append((sw, c - sw * SW_W, hi - c, c - b0))
            c = hi
        slot_pieces.append((j, TL, pieces))
    end_sweep = [(bases[j] + TL - 1) // SW_W if TL > 0 else 0
                 for j, TL, _ in slot_pieces]

    h_tiles = [None] * nsweep
    emitted = [False] * BC
    pending = []   # mm2 batches deferred one sweep for evac slack
    par = 0

    def flush_ready(upto_sweep, max_batches=None):
        nonlocal par
        if KNOBS["no_mm2"]:
            return
        ready = []
        for j in range(BC):
            if not emitted[j] and end_sweep[j] < upto_sweep:
                if all(h_tiles[sw] is not None
                       for (sw, _, _, _) in slot_pieces[j][2]):
                    ready.append(j)
        batches, batch, npieces = [], [], 0
        for j in ready:
            _, TL, pieces = slot_pieces[j]
            if npieces + len(pieces) > GRP:
                batches.append(batch)
                batch, npieces = [], 0
            batch.append((j, TL, [(h_tiles[sw], loc, n, oo)
                                  for (sw, loc, n, oo) in pieces]))
            npieces += len(pieces)
        if batch:
            batches.append(batch)
        if max_batches is not None:
            batches = batches[:max_batches]
        for b in batches:
            _emit_mm2_batch(nc, b, scps_p, stage_p, w2_sb, scores_all, par)
            par += len(b)
            for (j, _, _) in b:
                emitted[j] = True

    junk_ps = None
    if mode == "dr" and KNOBS["junk"]:
        junk_ps = junk_p.tile([128, 512], F32, tag="junkps")

    for sw in range(nsweep):
        xt = xts[sw]
        t0 = sw * SWEEP_T
        tiles = [(t, TILE_W * t - sw * SW_W,
                  min(TILE_W, P - TILE_W * t))
                 for t in range(t0, min(t0 + SWEEP_T, ntile))]
        h_sw = h_p.tile([128, 4, CW], BF16, tag="h")
        h_tiles[sw] = h_sw

        # weight-major: each stationary operand streams over the whole
        # sweep in one run, so every in-flight DR matmul is followed only
        # by a reload of its OWN weights (harmless) -- except the last of
        # the run, which a junk matmul protects from the next weights.
        hps = [None] * len(tiles)
        for g in range(4):
            for p in range(2):
                if mode == "dr":
                    w_ap = w1_sb[:, p * 8 + 2 * g:p * 8 + 2 * g + 2, :]
                    for i, (t, lo, n) in enumerate(tiles):
                        if p == 0 and hps[i] is None:
                            hp_new = hps_p.tile([128, 512], F32, tag="hps")
                            hps[i] = hp_new
                        nc.tensor.matmul(
                            hps[i][:, 0:n], w_ap,
                            xt[:, :, lo + p:lo + p + n],
                            start=(p == 0), stop=(p == 1),
                            perf_mode=DR)
                    if junk_ps is not None:
                        jn = KNOBS["junk_n"]
                        nc.tensor.matmul(junk_ps[:, 0:jn], w_ap,
                                         xt[:, :, 0:jn],
                                         start=True, stop=True,
                                         perf_mode=DR,
                                         skip_group_check=True)
                else:
                    for e in range(4):
                        w_ap = w1_sb[:, e * 4 + g, :]
                        if (e & 1) != p:
                            continue
                        for i, (t, lo, n) in enumerate(tiles):
                            if e == 0 and hps[i] is None:
                                hp_new = hps_p.tile([128, 512], F32,
                                                    tag="hps")
                                hps[i] = hp_new
                            nc.tensor.matmul(
                                hps[i][:, 0:n], w_ap,
                                xt[:, e & 1, lo + (e >> 1):lo + (e >> 1) + n],
                                start=(e == 0), stop=(e == 3))
                # evacuate after the stop pass: relu psum -> packed h bf16
                if p == 1:
                    for i, (t, lo, n) in enumerate(tiles):
                        hp = hps[i]
                        hps[i] = None
                        dst = h_sw[:, g, lo:lo + n]
                        if zero_b1:
                            if (sw + g + i) % 2 == 0:
                                nc.scalar.activation(dst, hp[:, 0:n], AF.Relu)
                            else:
                                nc.vector.tensor_scalar_max(dst, hp[:, 0:n],
                                                            0.0)
                        else:
                            if (sw + g + i) % 2 == 0:
                                nc.scalar.activation(dst, hp[:, 0:n], AF.Relu,
                                                     bias=b1_sb[:, g:g + 1],
                                                     scale=1.0)
                            else:
                                nc.vector.tensor_scalar(dst, hp[:, 0:n],
                                                        b1_sb[:, g:g + 1],
                                                        0.0, op0=ALU.add,
                                                        op1=ALU.max)
        flush_ready(sw)
    flush_ready(nsweep + 1)


def _emit_softmax(nc, sm_p, scores_all, mb_sb, oh_sb, out_d):
    logits = sm_p.tile([BC, S], F32)
    nc.vector.tensor_add(logits, scores_all, mb_sb)
    rowmax = sm_p.tile([BC, 1], F32)
    nc.vector.reduce_max(rowmax, logits, axis=AX.X)
    zt = sm_p.tile([BC, S], F32)
    nc.vector.tensor_scalar_sub(zt, logits, rowmax)
    et = sm_p.tile([BC, S], F32)
    sumexp = sm_p.tile([BC, 1], F32)
    nc.scalar.activation(et, zt, AF.Exp, accum_out=sumexp)
    logsum = sm_p.tile([BC, 1], F32)
    nc.scalar.activation(logsum, sumexp, AF.Ln)
    rinv = sm_p.tile([BC, 1], F32)
    nc.vector.reciprocal(rinv, sumexp)
    logp = sm_p.tile([BC, S], F32)
    nc.vector.tensor_scalar_sub(logp, zt, logsum)

    scr0 = sm_p.tile([BC, S], F32)
    lp = sm_p.tile([BC, 1], F32)
    nc.vector.tensor_mul(scr0, logp, oh_sb)
    nc.vector.reduce_sum(lp, scr0, axis=AX.X)
    scr1 = sm_p.tile([BC, S], F32)
    ez = sm_p.tile([BC, 1], F32)
    nc.vector.tensor_mul(scr1, et, zt)
    nc.vector.reduce_sum(ez, scr1, axis=AX.X)
    # entropy = logsum - (sum e*z) / sumexp
    ent = sm_p.tile([BC, 1], F32)
    nc.vector.tensor_mul(ent, ez, rinv)
    nc.vector.tensor_sub(ent, logsum, ent)

    res = sm_p.tile([BC, 2], F32)
    nc.vector.tensor_copy(res[:, 0:1], lp)
    nc.vector.tensor_copy(res[:, 1:2], ent)
    nc.sync.dma_start(out=out_d[:, :], in_=res)


# --------------------------------------------------------------------------
def prepare(x, W1, b1, W2, b2, lengths, position_action):
    """Host-side sharding: returns (slot_len, in_maps, core_rows)."""
    x = np.asarray(x, np.float32)
    W1 = np.asarray(W1, np.float32)
    b1 = np.asarray(b1, np.float32)
    W2 = np.asarray(W2, np.float32)
    lengths = np.asarray(lengths)
    position_action = np.asarray(position_action)
    mode = KNOBS["mode"]
    np_dt = NP_FP8 if mode == "dr" else NP_BF16
    wscale = W1SCALE if mode == "dr" else 1.0

    # length-sorted round-robin assignment: rank r -> core r%8, slot r//8
    order = np.argsort(lengths, kind="stable")
    slot_len = [int(lengths[order[j * N_CORES + N_CORES - 1]])
                for j in range(BC)]
    bases, P, ntile, nsweep = _geom(slot_len)

    # W1 chunk-pair layout [feat128, p*8+g*2+half, 128]; fp8 pre-scaled
    w1c = np.zeros((128, 16, 128), np_dt)
    W1s = (W1 * wscale).astype(np_dt)
    for p in range(2):
        for g in range(4):
            for half in range(2):
                blk = W1s[(2 * p + half) * 128:(2 * p + half + 1) * 128,
                          g * 128:(g + 1) * 128]
                if mode == "dr":
                    w1c[:, p * 8 + g * 2 + half, :] = blk
                else:
                    # bf16 path uses flat e*4+g chunks, e = 2*p+half
                    w1c[:, (2 * p + half) * 4 + g, :] = blk
    w2c = np.ascontiguousarray(
        (W2[:, 0] / wscale).reshape(4, 128).T).astype(NP_BF16)
    b1c = np.ascontiguousarray((b1 * wscale).reshape(4, 128).T
                               ).astype(np.float32)

    xq = x.astype(np_dt)  # (B, S, E)

    tcol = np.arange(S, dtype=np.int64)[None, :]
    in_maps, core_rows = [], []
    for core in range(N_CORES):
        rows = order[np.arange(BC) * N_CORES + core]
        core_rows.append(rows)
        # packed planes (128, 2, P+1), then chunked per sweep
        planes = np.zeros((128, 2, nsweep * SW_W + 1), np_dt)
        for j in range(BC):
            L = slot_len[j]
            r = rows[j]
            planes[:, 0, bases[j]:bases[j] + L] = xq[r, 0:L, 0:128].T
            planes[:, 1, bases[j]:bases[j] + L] = xq[r, 0:L, 128:256].T
        xp = np.zeros((nsweep, 128, 2, CW), np_dt)
        for sw in range(nsweep):
            w = min(SW_W + 1, planes.shape[2] - sw * SW_W)
            xp[sw, :, :, 0:w] = planes[:, :, sw * SW_W:sw * SW_W + w]

        lens = lengths[rows].astype(np.int64)
        mb = np.where(tcol < (lens - 1)[:, None],
                      np.float32(0), np.float32(NEG)).astype(np.float32)
        oh = np.zeros((BC, S), np.float32)
        oh[np.arange(BC), position_action[rows].astype(np.int64)] = 1.0
        in_maps.append({
            "x": xp,
            "w1": w1c, "w2": w2c, "b1": b1c,
            "maskbias": mb, "onehot": oh,
        })
    return slot_len, in_maps, core_rows


_prog_cache = {}
LAST_RESULT = None


def kernel(x, W1, b1, W2, b2, lengths, position_action):
    slot_len, in_maps, core_rows = prepare(
        x, W1, b1, W2, b2, lengths, position_action)

    zero_b1 = bool(np.all(np.asarray(b1) == 0))
    key = (tuple(slot_len), zero_b1, KNOBS["mode"])
    if key not in _prog_cache:
        _prog_cache[key] = _build_program(slot_len, zero_b1)
    nc = _prog_cache[key]

    br = run_bass_kernel_spmd(nc, in_maps, list(range(N_CORES)))
    global LAST_RESULT
    LAST_RESULT = br

    out = np.zeros((B, 2), np.float32)
    for core in range(N_CORES):
        out[core_rows[core]] = br.results[core]["out"]
    return out
